# revision 34
# baseline (speedup 1.0000x reference)
"""Trainium2 Bass kernel for nn_MHSA_CGLU (PSA attention + Convolutional GLU).

Sharding: data-parallel over batch (B=8), one NeuronCore per batch element.
Activations in [channels, N=H*W] layout (channels on SBUF partitions).

v3 structure (over the v2 baseline):
- x shipped as bf16 (halves input DMA, drops on-chip casts)
- conv biases folded into psum->SBUF copies (activation Identity with a
  per-partition bias column) instead of rank-1 bias matmuls
- dwconv diagonal tap matrices precomputed on host (GPSIMD freed)
- N=1024 moving operands everywhere (half the matmul/LDW instruction count)
- attention S/exp/O tiles are head-pure [128,1024]; exp split
  Scalar(table)/DVE(Schraudolph bit-trick) rebalanced ~9:7
- keep-warm dummies: short burst at startup to pre-warm the PE HAM clock
  gate + dep-staged dummies threaded through the LN/gelu serial chains
- activation table sets reordered so ln+exp share one table load
- vT softmax-ones column via GPSIMD memset instead of rank-1 matmuls
- fc2 tail pipelined per M-tile (yt add + output DMA overlap last matmuls)
"""

import functools

import ml_dtypes
import numpy as np

import concourse.bass as bass  # noqa: F401
import concourse.mybir as mybir
import concourse.tile as tile
from concourse import bacc
from concourse.bass_utils import run_bass_kernel_spmd

F32 = mybir.dt.float32
F32R = mybir.dt.float32r
BF16 = mybir.dt.bfloat16
I16 = mybir.dt.int16
U32 = mybir.dt.uint32
AF = mybir.ActivationFunctionType
OP = mybir.AluOpType

EPS = 1e-5
NH, KD, HD = 8, 16, 32
C, N, HH, WW = 256, 1024, 32, 32
HID = 170
SCALE = KD ** -0.5

# Schraudolph exp -> bf16 bits via int16: round(x*EC1 + EC2)
EC1 = float(np.log2(np.e) * 128.0)
EC2 = float(127.0 * 128.0 - 4.7)


# --------------------------------------------------------------------------
# Host-side parameter folding
# --------------------------------------------------------------------------

def _bn_fold(p):
    g, b, m, v = [np.asarray(a, np.float64) for a in p]
    s = g / np.sqrt(v + EPS)
    return s, b - s * m


# (name, rows, free-shape) laid out contiguously in the bf16 blob.
# Early group (LN1/qkv/vT) first; tail weights + diag tap matrices second.
BLOB_SLOTS = [
    ("statcol33", 128, [2, 33]), ("ones128", 128, [128]),
    ("wqkvT", 128, [2, 768]), ("wvT", 128, [2, 264]),
    # ---- EARLY_COLS boundary ----
    ("id128", 128, [128]),
    ("wprojT", 128, [2, 256]), ("wfc1T", 128, [2, 512]),
    ("wfc2T", 128, [2, 256]),
    ("diags", 128, [36, 128]),
]
EARLY_COLS = 66 + 128 + 1536 + 528
BLOB_COLS = sum(int(np.prod(sh)) for _, _, sh in BLOB_SLOTS)

# rows8 [8, ...]: ind (8 rows), ones_row / biasG (row 0)
ROWS8_SLOTS = [("ind", 8, [256]), ("ones_row", 1, [1024]), ("biasG", 1, [256])]
ROWS8_COLS = sum(int(np.prod(sh)) for _, _, sh in ROWS8_SLOTS)

# cols [128, NCOL] f32: per-partition columns
COL_IDX = {
    "epscol": 0,
    "bdw0": 1, "bdw1": 2,          # gelu bias (dw_b)
    "g2c0": 3, "g2c1": 4,          # ln2_g (xn2 scale)
    "bfin0": 5, "bfin1": 6,        # fc2_b + ln2_b
    "bq0": 7, "bq1": 8,            # qkv bias, Q tiles
    "bk0": 9, "bk1": 10,           # qkv bias, K tiles
    "bv0": 11, "bv1": 12,          # qkv bias, V tiles
    "bA0": 13, "bA1": 14,          # fc1 bias, A tiles
    "bpr0": 15, "bpr1": 16,        # proj bias
}
NCOL = 17


def fold_consts(inp):
    f64 = lambda a: np.asarray(a, np.float64)
    ln1_g, ln1_b = f64(inp["ln1_g"]), f64(inp["ln1_b"])
    ln2_g, ln2_b = f64(inp["ln2_g"]), f64(inp["ln2_b"])

    # qkv conv + BN, with LN1 affine folded in.
    s_qkv, b_qkv = _bn_fold(inp["qkv_bn"])
    Wq = s_qkv[:, None] * f64(inp["qkv_w"])          # [512, 256]
    bq = b_qkv.copy()
    bq += Wq @ ln1_b
    Wq = Wq * ln1_g[None, :]

    q_rows = np.concatenate([np.arange(64 * h, 64 * h + 16) for h in range(NH)])
    k_rows = q_rows + 16
    v_rows = np.concatenate([np.arange(64 * h + 32, 64 * h + 64) for h in range(NH)])
    Wq_q, bq_q = Wq[q_rows] * SCALE, bq[q_rows] * SCALE
    Wq_k, bq_k = Wq[k_rows], bq[k_rows]
    Wq_v, bq_v = Wq[v_rows], bq[v_rows]

    # qkv M-tiles: Q0(h0-3), Q1(h4-7), K0, K1 (head j at rows 32j..32j+16,
    # rest zero), V0, V1 dense. Biases as per-partition columns.
    Wfull = np.zeros((6, 128, 256))
    bias_cols = np.zeros((128, NCOL))
    for h in range(NH):
        T, j = divmod(h, 4)
        sl = slice(32 * j, 32 * j + 16)
        Wfull[T][sl] = Wq_q[16 * h: 16 * h + 16]
        bias_cols[sl, COL_IDX[f"bq{T}"]] = bq_q[16 * h: 16 * h + 16]
        Wfull[2 + T][sl] = Wq_k[16 * h: 16 * h + 16]
        bias_cols[sl, COL_IDX[f"bk{T}"]] = bq_k[16 * h: 16 * h + 16]
    Wfull[4] = Wq_v[0:128]
    Wfull[5] = Wq_v[128:256]
    bias_cols[:, COL_IDX["bv0"]] = bq_v[0:128]
    bias_cols[:, COL_IDX["bv1"]] = bq_v[128:256]
    # SBUF layout [part(cin%128), kt(cin//128), 6*128 m-cols]
    wqkvT = np.ascontiguousarray(
        Wfull.reshape(768, 256).T.reshape(2, 128, 768).transpose(1, 0, 2))

    # v^T conv: [n, 33h+d]; col 33h+32 is the softmax-ones column (zero
    # weight; ones written on-device by GPSIMD memset).
    WvT = np.zeros((256, 264))
    for h in range(NH):
        WvT[:, 33 * h: 33 * h + 32] = Wq_v[32 * h: 32 * h + 32].T
    wvT = np.ascontiguousarray(WvT.reshape(2, 128, 264).transpose(1, 0, 2))

    # pe branch dwconv taps (BN scale folded); o2 + bq_v + b_pe folded
    # through proj into its bias.
    s_pe, b_pe = _bn_fold(inp["pe_bn"])
    taps_pe = s_pe[:, None, None] * f64(inp["pe_w"])[:, 0]     # [256, 3, 3]
    bfold_pe = b_pe + bq_v

    s_pr, b_pr = _bn_fold(inp["proj_bn"])
    Wpr = s_pr[:, None] * f64(inp["proj_w"])
    bias_proj = b_pr + Wpr @ bfold_pe
    bias_cols[:, COL_IDX["bpr0"]] = bias_proj[0:128]
    bias_cols[:, COL_IDX["bpr1"]] = bias_proj[128:256]
    wprojT = np.ascontiguousarray(Wpr.T.reshape(2, 128, 256).transpose(1, 0, 2))

    # fc1 with LN2 affine folded; M-tiles A1(128) A2(42) G1(128) G2(42)
    W1 = f64(inp["fc1_w"])
    b1 = f64(inp["fc1_b"]) + W1 @ ln2_b
    W1 = W1 * ln2_g[None, :]
    W1cols = np.zeros((256, 512))
    W1cols[:, 0:128] = W1[0:128].T
    W1cols[:, 128:170] = W1[128:170].T
    W1cols[:, 256:384] = W1[170:298].T
    W1cols[:, 384:426] = W1[298:340].T
    bias_cols[0:128, COL_IDX["bA0"]] = b1[0:128]
    bias_cols[0:42, COL_IDX["bA1"]] = b1[128:170]
    biasG = np.zeros((1, 256))
    biasG[0, 0:128] = b1[170:298]
    biasG[0, 128:170] = b1[298:340]
    wfc1T = np.ascontiguousarray(W1cols.reshape(2, 128, 512).transpose(1, 0, 2))

    taps_dw = f64(inp["dw_w"])[:, 0]                            # [170, 3, 3]
    b_dw = f64(inp["dw_b"])
    bias_cols[0:128, COL_IDX["bdw0"]] = b_dw[0:128]
    bias_cols[0:42, COL_IDX["bdw1"]] = b_dw[128:170]

    W2 = f64(inp["fc2_w"])                                      # [256, 170]
    W2T = np.zeros((2, 128, 256))
    W2T[0] = W2[:, 0:128].T
    W2T[1, 0:42] = W2[:, 128:170].T
    wfc2T = np.ascontiguousarray(W2T.transpose(1, 0, 2))        # [128, 2, 256]
    bfin = f64(inp["fc2_b"]) + ln2_b
    bias_cols[:, COL_IDX["bfin0"]] = bfin[0:128]
    bias_cols[:, COL_IDX["bfin1"]] = bfin[128:256]
    bias_cols[:, COL_IDX["g2c0"]] = ln2_g[0:128]
    bias_cols[:, COL_IDX["g2c1"]] = ln2_g[128:256]
    bias_cols[:, COL_IDX["epscol"]] = EPS

    # host-precomputed diag tap matrices [128, 36, 128]:
    # index t*9+tap; t=0,1 -> pe (channels 0:128 / 128:256),
    # t=2,3 -> glu dw (channels 0:128 / 128:170)
    diags = np.zeros((128, 36, 128))
    rng = np.arange(128)
    for tap in range(9):
        dy, dx = divmod(tap, 3)
        diags[rng, 0 * 9 + tap, rng] = taps_pe[0:128, dy, dx]
        diags[rng, 1 * 9 + tap, rng] = taps_pe[128:256, dy, dx]
        diags[rng, 2 * 9 + tap, rng] = taps_dw[0:128, dy, dx]
        r42 = np.arange(42)
        diags[r42, 3 * 9 + tap, r42] = taps_dw[128:170, dy, dx]

    ind = np.zeros((8, 256))
    for h in range(NH):
        ind[h, 32 * h: 32 * h + 32] = 1.0

    # stat lhsT columns, replicated to M=33 so the psum stat rows 0..32 are
    # all written (rows 1..31 are dummies; row 0 = chunk0, row 32 = chunk1)
    statcol33 = np.zeros((128, 2, 33))
    statcol33[:, 0, :] = -1.0 / C
    statcol33[:, 1, :] = 1.0 / C

    # ---- bf16 blob [128, BLOB_COLS] ----
    blob = np.zeros((128, BLOB_COLS))
    arrs = {
        "statcol33": statcol33.reshape(128, -1),
        "ones128": np.ones((128, 128)),
        "wqkvT": wqkvT.reshape(128, -1),
        "wvT": wvT.reshape(128, -1),
        "id128": np.eye(128),
        "wprojT": wprojT.reshape(128, -1),
        "wfc1T": wfc1T.reshape(128, -1),
        "wfc2T": wfc2T.reshape(128, -1),
        "diags": diags.reshape(128, -1),
    }
    off = 0
    for nm, rows, sh in BLOB_SLOTS:
        a = arrs[nm]
        c = a.shape[1]
        assert c == int(np.prod(sh)), (nm, c, sh)
        blob[0:rows, off:off + c] = a
        off += c
    assert off == BLOB_COLS, off

    rows8 = np.zeros((8, ROWS8_COLS))
    r_arrs = {"ind": ind, "ones_row": np.ones((1, 1024)), "biasG": biasG}
    off = 0
    for nm, rows, sh in ROWS8_SLOTS:
        a = r_arrs[nm]
        c = a.shape[1]
        rows8[0:rows, off:off + c] = a
        off += c

    f32 = lambda a: np.ascontiguousarray(a, dtype=np.float32)
    bf16 = lambda a: np.ascontiguousarray(a, dtype=ml_dtypes.bfloat16)
    return {
        "blob": bf16(blob),
        "rows8": bf16(rows8),
        "cols": f32(bias_cols),
    }


def pack_x(xb):
    """[C, N] f32 -> [128, 2*N] bf16 (tile t at cols t*N..t*N+N)."""
    return np.ascontiguousarray(
        np.asarray(xb, np.float32).reshape(2, 128, N).transpose(1, 0, 2)
        .reshape(128, 2 * N), dtype=ml_dtypes.bfloat16)


# --------------------------------------------------------------------------
# Device program (one core, one batch)
# --------------------------------------------------------------------------

def _ln(nc, work, rows, psS, psO, xb, consts, z_tiles, sdummy, gp_half=False):
    """LayerNorm over channels. xb: 2x[128,N] bf16 input tiles.
    Writes z_tiles (bf16): z = (x - mu) * rstd."""
    xsq = [work.tile([128, N], BF16, tag=f"xsq{t}", name=f"xsq{t}") for t in range(2)]
    for t in range(2):
        nc.vector.tensor_tensor(xsq[t][:], xb[t][:], xb[t][:], OP.mult)

    # stats psum tile: bank0 = -mean rows, bank1 = E[x^2] rows: chunk c0 via
    # M=33 matmul (rows 0..32 all written = valid), chunk c1 overwrites row 32.
    sp = psO.tile([128, N], F32, tag="psO", name="ln_stats")
    mcol33 = consts["statcol33"][:, 0, :]
    ecol33 = consts["statcol33"][:, 1, :]
    for t in range(2):
        nc.tensor.matmul(sp[0:33, 0:512], mcol33[:], xb[t][:, 0:512],
                         start=(t == 0), stop=(t == 1))
    for t in range(2):
        nc.tensor.matmul(sp[32:33, 0:512], mcol33[:, 0:1], xb[t][:, 512:1024],
                         start=(t == 0), stop=(t == 1))
    for t in range(2):
        nc.tensor.matmul(sp[0:33, 512:1024], ecol33[:], xsq[t][:, 0:512],
                         start=(t == 0), stop=(t == 1))
    for t in range(2):
        nc.tensor.matmul(sp[32:33, 512:1024], ecol33[:, 0:1], xsq[t][:, 512:1024],
                         start=(t == 0), stop=(t == 1))

    # row math on [33, 512]: rows 0 (chunk0) and 32 (chunk1) are live.
    msb = rows.tile([33, 512], F32R, tag="msb", name="ln_msb")
    nc.vector.tensor_copy(msb[:], sp[0:33, 0:512])          # -mu
    mu2 = rows.tile([33, 512], F32R, tag="mu2", name="ln_mu2")
    nc.vector.tensor_tensor(mu2[:], msb[:], msb[:], OP.mult)
    var = rows.tile([33, 512], F32R, tag="var", name="ln_var")
    nc.vector.tensor_tensor(var[:], sp[0:33, 512:1024], mu2[:], OP.subtract)
    sdummy(mu2[:, 0:128], mu2[:, 0:512])
    nc.scalar.activation(var[:], var[:], AF.Ln,
                         bias=consts["cols"][0:33, COL_IDX["epscol"]:COL_IDX["epscol"] + 1])
    A = rows.tile([33, 512], BF16, tag="A", name="ln_A")
    nc.scalar.activation(A[:], var[:], AF.Exp, scale=-0.5)  # rstd
    sdummy(var[:, 0:128], var[:, 0:512])
    Br = rows.tile([33, 512], BF16, tag="Br", name="ln_Br")
    nc.vector.tensor_tensor(Br[:], msb[:], A[:], OP.mult)   # -mu*rstd

    # broadcast per chunk: bc = [A_c | Br_c] in one psum tile
    ones = consts["ones128"]
    absb = []
    for c in range(2):
        r = 32 * c
        bc = psS.tile([128, N], F32, tag="psS", name=f"ln_bc{c}")
        nc.tensor.matmul(bc[:, 0:512], ones[r:r + 1, 0:128], A[r:r + 1, :],
                         start=True, stop=True)
        nc.tensor.matmul(bc[:, 512:1024], ones[r:r + 1, 0:128], Br[r:r + 1, :],
                         start=True, stop=True)
        Ac = work.tile([128, 512], BF16, tag=f"Ac{c}", name=f"ln_Ac{c}")
        Bc = work.tile([128, 512], BF16, tag=f"Bc{c}", name=f"ln_Bc{c}")
        nc.scalar.copy(Ac[:], bc[:, 0:512])
        nc.scalar.copy(Bc[:], bc[:, 512:1024])
        absb.append((Ac, Bc))
        sdummy(A[:, 0:128], Br[:, 0:512])

    for t in range(2):
        for c in range(2):
            sl = slice(512 * c, 512 * c + 512)
            Ac, Bc = absb[c]
            eng = nc.gpsimd if (gp_half and t == 1) else nc.vector
            eng.tensor_tensor(z_tiles[t][:, sl], xb[t][:, sl], Ac[:], OP.mult)
            eng.tensor_tensor(z_tiles[t][:, sl], z_tiles[t][:, sl], Bc[:], OP.add)


GELU_AF = None  # set to AF.Tanh for CoreSim debug (Gelu unimplemented there)


def build(num_devices=8, debug_outs=False):
    gelu_af = GELU_AF or AF.Gelu
    nc = bacc.Bacc("TRN2", target_bir_lowering=False, debug=False,
                   num_devices=num_devices)

    x_d = nc.dram_tensor("x", [128, 2 * N], BF16, kind="ExternalInput")
    blob_d = nc.dram_tensor("blob", [128, BLOB_COLS], BF16, kind="ExternalInput")
    rows8_d = nc.dram_tensor("rows8", [8, ROWS8_COLS], BF16, kind="ExternalInput")
    cols_d = nc.dram_tensor("cols", [128, NCOL], F32, kind="ExternalInput")
    y_d = nc.dram_tensor("y", [C, N], F32, kind="ExternalOutput")
    dbg = {}
    if debug_outs:
        for nm, sh, dt in [("d_z1", [128, N], BF16), ("d_q0", [128, N], BF16),
                           ("d_k0", [128, N], BF16), ("d_v0", [128, 34, 36], BF16),
                           ("d_vt0", [128, 8, 33], BF16),
                           ("d_pt", [128, N], BF16), ("d_oall0", [128, N], BF16),
                           ("d_r128", [128, 64], BF16), ("d_o20", [128, N], BF16),
                           ("d_xa0", [128, N], BF16), ("d_z20", [128, N], BF16),
                           ("d_a0", [128, 34, 36], BF16), ("d_ag0", [128, N], BF16),
                           ("d_pe0", [128, N], BF16)]:
            dbg[nm] = nc.dram_tensor(nm, sh, dt, kind="ExternalOutput")

    with tile.TileContext(nc) as tc:
        with tc.tile_pool(name="singles", bufs=1) as singles, \
             tc.tile_pool(name="work", bufs=1) as work, \
             tc.tile_pool(name="rows", bufs=2) as rows, \
             tc.tile_pool(name="ptp", bufs=34) as ptp, \
             tc.tile_pool(name="stg", bufs=2) as stg, \
             tc.tile_pool(name="psS", bufs=2, space="PSUM") as psS, \
             tc.tile_pool(name="psO", bufs=1, space="PSUM") as psO, \
             tc.tile_pool(name="psD", bufs=1, space="PSUM") as psD:

            # ---- garbage tile for HAM warm-up dummies (memset first so
            # nothing reads uninitialized SBUF) ----
            garb = singles.tile([128, 512], BF16, tag="garb", name="garb")
            nc.gpsimd.memset(garb[:].bitcast(U32), 0)

            # ---- input + constants; first-needed pieces on fast queues ----
            xbt = singles.tile([128, 2 * N], BF16, tag="x", name="x")
            nc.sync.dma_start(xbt[:, 0:N], x_d.ap()[:, 0:N])
            nc.scalar.dma_start(xbt[:, N:2 * N], x_d.ap()[:, N:2 * N])
            xts = [xbt[:, 0:N], xbt[:, N:2 * N]]
            blob = singles.tile([128, BLOB_COLS], BF16, tag="blob", name="blob")
            h = EARLY_COLS // 2
            nc.sync.dma_start(blob[:, 0:h], blob_d.ap()[:, 0:h])
            nc.scalar.dma_start(blob[:, h:EARLY_COLS], blob_d.ap()[:, h:EARLY_COLS])
            cols_t = singles.tile([128, NCOL], F32, tag="cols", name="cols")
            nc.scalar.dma_start(cols_t[:], cols_d.ap())
            rows8_t = singles.tile([8, ROWS8_COLS], BF16, tag="rows8", name="rows8")
            nc.scalar.dma_start(rows8_t[:], rows8_d.ap())
            nc.gpsimd.dma_start(blob[:, EARLY_COLS:], blob_d.ap()[:, EARLY_COLS:])

            consts = {"cols": cols_t}
            _off = 0
            for _nm, _rows, _sh in BLOB_SLOTS:
                _c = int(np.prod(_sh))
                _v = blob[0:_rows, _off:_off + _c]
                if len(_sh) == 2:
                    _v = _v.rearrange("p (a b) -> p a b", a=_sh[0])
                consts[_nm] = _v
                _off += _c
            _off = 0
            for _nm, _rows, _sh in ROWS8_SLOTS:
                _c = int(np.prod(_sh))
                consts[_nm] = rows8_t[0:_rows, _off:_off + _c]
                _off += _c

            def col(name):
                i = COL_IDX[name]
                return consts["cols"][:, i:i + 1]

            # activation-table prewarm (Ln+Exp share a set; Gelu its own).
            actw = work.tile([1, 4], F32, tag="actw", name="actw")
            nc.scalar.activation(actw[0:1, 0:1], garb[0:1, 0:1], AF.Ln, bias=1.0)
            nc.scalar.activation(actw[0:1, 1:2], garb[0:1, 0:1], AF.Exp)
            nc.scalar.activation(actw[0:1, 2:3], garb[0:1, 0:1], gelu_af)

            # HAM warm-up: ~6 cold dummies span the ~3.4us activity window
            # so LN1/qkv matmuls run at 2.4 GHz.
            dumref = [psD.tile([128, 1024], F32, tag="psD", name="dum")]

            def emit_dummy(k, n=512):
                for _ in range(k):
                    nc.tensor.matmul(dumref[0][:, 0:n], garb[:, 0:128],
                                     garb[:, 0:n], start=True, stop=True,
                                     skip_group_check=True)

            def sdummy(lhsT, rhs):
                # dep-staged keep-warm dummy: fires when `rhs` is written
                nc.tensor.matmul(dumref[0][:, 0:rhs.free_size()], lhsT, rhs,
                                 start=True, stop=True, skip_group_check=True)

            emit_dummy(6)

            # padded dwconv inputs [128, 34, 36]; interior rows 1:33, cols 2:34
            vpad = [work.tile([128, 34, 36], BF16, tag=f"vpad{t}", name=f"vpad{t}")
                    for t in range(2)]
            apad = [work.tile([128, 34, 36], BF16, tag=f"apad{t}", name=f"apad{t}")
                    for t in range(2)]
            for t in range(2):
                nc.gpsimd.memset(vpad[t][:].bitcast(U32), 0)
                nc.gpsimd.memset(apad[t][:].bitcast(U32), 0)
            recip_row = work.tile([8, N], BF16, tag="recip_row", name="recip_row")
            nc.gpsimd.memset(recip_row[:].bitcast(U32), 0)

            diags = consts["diags"]  # [128, 36, 128]; index t*9+tap

            # ---- LN1 ----
            z1 = [work.tile([128, N], BF16, tag=f"z1_{t}", name=f"z1_{t}") for t in range(2)]
            _ln(nc, work, rows, psS, psO, xts, consts, z1, sdummy)

            # ---- qkv conv: M-tiles Q0 Q1 K0 K1 V0 V1 (N=1024 moving) ----
            qk_sb = []
            for mt in range(6):
                ps = psS.tile([128, N], F32, tag="psS", name=f"qkv{mt}")
                for c in range(2):
                    sl = slice(c * 512, (c + 1) * 512)
                    for kt in range(2):
                        nc.tensor.matmul(
                            ps[:, sl], consts["wqkvT"][:, kt, mt * 128:(mt + 1) * 128],
                            z1[kt][:, sl], start=(kt == 0), stop=(kt == 1))
                if mt < 2:       # Q tiles: Scalar copy + bias column
                    t_sb = work.tile([128, N], BF16, tag=f"qk{mt}", name=f"qk{mt}")
                    nc.scalar.activation(t_sb[:], ps[:], AF.Identity,
                                         bias=col(f"bq{mt}"))
                    qk_sb.append(t_sb)
                elif mt < 4:     # K tiles: DVE add bias column
                    t_sb = work.tile([128, N], BF16, tag=f"qk{mt}", name=f"qk{mt}")
                    nc.vector.tensor_scalar(t_sb[:], ps[:], col(f"bk{mt - 2}"),
                                            None, OP.add)
                    qk_sb.append(t_sb)
                else:            # V tiles -> padded dwconv input, + bias
                    nc.scalar.activation(vpad[mt - 4][:, 1:33, 2:34], ps[:],
                                         AF.Identity, bias=col(f"bv{mt - 4}"))
            q_sb, k_sb = qk_sb[0:2], qk_sb[2:4]

            # ---- v^T conv (softmax-ones column via memset) ----
            vT_sb = []
            for nt in range(8):
                ps = psS.tile([128, 264], F32, tag="psS", name=f"vT{nt}")
                for kt in range(2):
                    nc.tensor.matmul(
                        ps[:], z1[kt][:, nt * 128:(nt + 1) * 128],
                        consts["wvT"][:, kt, :], start=(kt == 0), stop=(kt == 1))
                t_sb = work.tile([128, 8, 33], BF16, tag=f"vT{nt}", name=f"vT{nt}")
                nc.vector.tensor_copy(t_sb[:], ps[:].rearrange("p (a b) -> p a b", a=8))
                nc.gpsimd.memset(t_sb[:, :, 32:33], 1.0)
                vT_sb.append(t_sb)

            # ---- attention (pipelined over head pairs) ----
            # pair p: heads (2p, 2p+1); head h: q/k tile h//4, row group 32*(h%4)
            pts = {}        # (p, mt, i) -> bf16 [128, N] head-pure exp tile
            o_all = [work.tile([128, N], BF16, tag=f"oall{t}", name=f"oall{t}")
                     for t in range(2)]
            o2e = [work.tile([128, N], BF16, tag=f"o2{t}", name=f"o2{t}")
                   for t in range(2)]
            r128 = work.tile([128, 64], BF16, tag="r128", name="r128")
            pe_sb = [work.tile([128, N], BF16, tag=f"pe{t}", name=f"pe{t}")
                     for t in range(2)]
            o_ps = {}

            def emit_s_exp(p, mt):
                T = p // 2
                msl = slice(mt * 128, (mt + 1) * 128)
                sms = [psS.tile([128, N], F32, tag="psS", name=f"s{p}_{mt}_{i}")
                       for i in range(2)]
                # zigzag (h0c0, h1c0, h1c1, h0c1): chunks of the two heads
                # overlap in the array (different row strips) and consecutive
                # same-head matmuls reuse the stationary.
                for i, c in ((0, 0), (1, 0), (1, 1), (0, 1)):
                    g = 32 * ((2 * p + i) % 4)
                    sl = slice(c * 512, (c + 1) * 512)
                    nc.tensor.matmul(sms[i][:, sl], k_sb[T][g:g + 16, msl],
                                     q_sb[T][g:g + 16, sl],
                                     start=True, stop=True, tile_position=(g, 0))
                for i in range(2):
                    pt = ptp.tile([128, N], BF16, tag="pt", name=f"pt{p}_{mt}_{i}")
                    # Scalar table-exp for head0 (+ head1 every 8th step to
                    # balance); DVE Schraudolph bit-trick otherwise.
                    if i == 0 or mt == 5:
                        nc.scalar.activation(pt[:], sms[i][:], AF.Exp)
                    else:
                        nc.vector.tensor_scalar(
                            pt[:].bitcast(I16), sms[i][:], EC1, EC2, OP.mult, OP.add)
                    pts[(p, mt, i)] = pt

            def emit_o(p, mt):
                if mt == 0:
                    o_ps[p] = psO.tile([128, N], F32, tag="psO", name=f"o{p}")
                ops = o_ps[p]
                for i, c in ((0, 0), (1, 0), (1, 1), (0, 1)):
                    base = 64 * i
                    h = 2 * p + i
                    sl = slice(c * 512, (c + 1) * 512)
                    nc.tensor.matmul(
                        ops[base:base + 33, sl], vT_sb[mt][:, h, :],
                        pts[(p, mt, i)][:, sl], start=(mt == 0), stop=(mt == 7),
                        tile_position=(0, base), skip_group_check=(i == 1))

            def emit_stage(p):
                h0, h1 = 2 * p, 2 * p + 1
                stage = stg.tile([97, N], BF16, tag="stage", name=f"stage{p}")
                for base in (0, 64):
                    if p == 3:
                        nc.scalar.copy(stage[base:base + 33, :],
                                       o_ps[p][base:base + 33, :])
                    else:
                        nc.vector.tensor_copy(stage[base:base + 33, :],
                                              o_ps[p][base:base + 33, :])
                for hh, base in ((h0, 0), (h1, 64)):
                    oT, oj = divmod(hh, 4)
                    nc.sync.dma_start(o_all[oT][32 * oj: 32 * oj + 32, :],
                                      stage[base: base + 32, :])
                    nc.sync.dma_start(r128[16 * hh:16 * hh + 16, :],
                                      stage[base + 32: base + 33, :])

            def emit_pe_dwconv(t):
                ps = psS.tile([128, N], F32, tag="psS", name=f"pe_ps{t}")
                for tap in range(9):
                    dy, dx = divmod(tap, 3)
                    for c in range(2):
                        rhs = vpad[t][:, dy + 16 * c: dy + 16 * c + 16, dx + 1: dx + 33]
                        nc.tensor.matmul(ps[:, c * 512:(c + 1) * 512],
                                         diags[:, t * 9 + tap, :], rhs,
                                         start=(tap == 0), stop=(tap == 8))
                nc.vector.tensor_copy(pe_sb[t][:], ps[:])

            def emit_recip_quarter(q):
                lo = 32 * q
                with nc.allow_low_precision(reason="softmax recip"):
                    nc.vector.reciprocal(recip128[lo:lo + 32, :], r128[lo:lo + 32, :])
                nc.sync.dma_start(recip_row[2 * q:2 * q + 2, :],
                                  recip128[lo:lo + 32, :])

            recip128 = work.tile([128, 64], BF16, tag="recip128", name="recip128")

            for p in range(4):
                for mt in range(8):
                    emit_s_exp(p, mt)
                    if p >= 1:
                        emit_o(p - 1, mt)
                if p >= 1:
                    emit_stage(p - 1)
                if p == 1:
                    emit_pe_dwconv(0)
                    emit_recip_quarter(0)   # heads 0,1 (stage 0 done)
                if p == 2:
                    emit_pe_dwconv(1)
                    emit_recip_quarter(1)   # heads 2,3
                if p == 3:
                    emit_recip_quarter(2)   # heads 4,5 (stage 2 done)
                    # heads 0-3 normalization completes during attention:
                    # recipB0 uses the (rotated) dummy psum bank.
                    rb0 = psD.tile([128, N], F32, tag="psD", name="recipB0")
                    dumref[0] = rb0
                    for c in range(2):
                        sl = slice(c * 512, (c + 1) * 512)
                        nc.tensor.matmul(rb0[:, sl], consts["ind"][:, 0:128],
                                         recip_row[:, sl], start=True, stop=True)
                    nc.vector.tensor_tensor(o2e[0][:], o_all[0][:], rb0[:], OP.mult)
                    nc.vector.tensor_tensor(o2e[0][:], o2e[0][:], pe_sb[0][:], OP.add)
                    dum2 = psD.tile([128, N], F32, tag="psD", name="dum2")
                    dumref[0] = dum2
            for mt in range(8):
                emit_o(3, mt)
            emit_stage(3)
            emit_recip_quarter(3)   # heads 6,7

            if debug_outs:
                nc.sync.dma_start(dbg["d_z1"].ap(), z1[0][:])
                nc.sync.dma_start(dbg["d_q0"].ap(), q_sb[0][:])
                nc.sync.dma_start(dbg["d_k0"].ap(), k_sb[0][:])
                nc.sync.dma_start(dbg["d_v0"].ap(), vpad[0][:])
                nc.sync.dma_start(dbg["d_vt0"].ap(), vT_sb[0][:])
                nc.sync.dma_start(dbg["d_pt"].ap(), pts[(0, 0, 0)][:])
                nc.sync.dma_start(dbg["d_oall0"].ap(), o_all[0][:])
                nc.sync.dma_start(dbg["d_r128"].ap(), r128[:])
                nc.sync.dma_start(dbg["d_pe0"].ap(), pe_sb[0][:])

            # ---- normalize + pe add for heads 4-7 (0-3 done in-attention) ----
            o2 = o2e
            rb = psS.tile([128, N], F32, tag="psS", name="recipB1")
            for c in range(2):
                sl = slice(c * 512, (c + 1) * 512)
                nc.tensor.matmul(rb[:, sl], consts["ind"][:, 128:256],
                                 recip_row[:, sl], start=True, stop=True)
            nc.vector.tensor_tensor(o2[1][:], o_all[1][:], rb[:], OP.mult)
            nc.vector.tensor_tensor(o2[1][:], o2[1][:], pe_sb[1][:], OP.add)
            sdummy(o2[1][:, 0:128], o2[1][:, 0:512])

            # ---- proj conv + residual; x_attn in bf16 ----
            x_attn = [work.tile([128, N], BF16, tag=f"xa{t}", name=f"xa{t}")
                      for t in range(2)]
            for mt in range(2):
                ps = psS.tile([128, N], F32, tag="psS", name=f"proj{mt}")
                for c in range(2):
                    sl = slice(c * 512, (c + 1) * 512)
                    for kt in range(2):
                        nc.tensor.matmul(
                            ps[:, sl], consts["wprojT"][:, kt, mt * 128:(mt + 1) * 128],
                            o2[kt][:, sl], start=(kt == 0), stop=(kt == 1))
                tmp = work.tile([128, N], BF16, tag=f"pj{mt}", name=f"pj{mt}")
                nc.scalar.activation(tmp[:], ps[:], AF.Identity, bias=col(f"bpr{mt}"))
                nc.vector.tensor_tensor(x_attn[mt][:], xts[mt], tmp[:], OP.add)
                sdummy(tmp[:, 0:128], tmp[:, 0:512])

            if debug_outs:
                nc.sync.dma_start(dbg["d_o20"].ap(), o2[0][:])
                nc.sync.dma_start(dbg["d_xa0"].ap(), x_attn[0][:])

            # ---- LN2 ----
            z2 = [work.tile([128, N], BF16, tag=f"z2_{t}", name=f"z2_{t}") for t in range(2)]
            _ln(nc, work, rows, psS, psO, x_attn, consts, z2, sdummy, gp_half=True)
            if debug_outs:
                nc.sync.dma_start(dbg["d_z20"].ap(), z2[0][:])

            # ---- fc1: M-tiles A1(128) A2(42) G1(128) G2(42) ----
            g_ps = []
            nparts = [128, 42, 128, 42]
            fc1_pools = [(psS, "psS"), (psS, "psS"), (psO, "psO"), (psS, "psS")]
            ones_row = consts["ones_row"]
            for mt in range(4):
                npart = nparts[mt]
                pool, tagname = fc1_pools[mt]
                ps = pool.tile([128, N], F32, tag=tagname, name=f"fc1_{mt}")
                for c in range(2):
                    sl = slice(c * 512, (c + 1) * 512)
                    for kt in range(2):
                        nc.tensor.matmul(
                            ps[0:npart, sl],
                            consts["wfc1T"][:, kt, mt * 128: mt * 128 + npart],
                            z2[kt][:, sl], start=(kt == 0), stop=(mt < 2 and kt == 1))
                    if mt >= 2:
                        nc.tensor.matmul(
                            ps[0:npart, sl],
                            consts["biasG"][:, (mt - 2) * 128:(mt - 2) * 128 + npart],
                            ones_row[:, 0:512], start=False, stop=True)
                if mt < 2:
                    nc.scalar.activation(apad[mt][0:npart, 1:33, 2:34], ps[0:npart],
                                         AF.Identity, bias=col(f"bA{mt}")[0:npart])
                else:
                    g_ps.append(ps)

            # ---- GLU dwconv + gelu + gate ----
            da_ps = [psS.tile([128, N], F32, tag="psS", name=f"da{t}")
                     for t in range(2)]
            for tap in range(9):
                dy, dx = divmod(tap, 3)
                for t in range(2):
                    npart = nparts[t]
                    for c in range(2):
                        rhs = apad[t][0:npart, dy + 16 * c: dy + 16 * c + 16,
                                      dx + 1: dx + 33]
                        nc.tensor.matmul(
                            da_ps[t][0:npart, c * 512:(c + 1) * 512],
                            diags[0:npart, (2 + t) * 9 + tap, 0:npart], rhs,
                            start=(tap == 0), stop=(tap == 8))
            ag = []
            for t in range(2):
                npart = nparts[t]
                a_act = work.tile([128, N], BF16, tag=f"aact{t}", name=f"aact{t}")
                nc.scalar.activation(a_act[0:npart], da_ps[t][0:npart], gelu_af,
                                     bias=col(f"bdw{t}")[0:npart])
                sdummy(a_act[0:npart, 0:128], a_act[0:npart, 0:512])
                agt = work.tile([128, N], BF16, tag=f"ag{t}", name=f"ag{t}")
                nc.vector.tensor_tensor(agt[0:npart], a_act[0:npart],
                                        g_ps[t][0:npart], OP.mult)
                ag.append(agt)
                sdummy(agt[0:npart, 0:128], agt[0:npart, 0:512])
            if debug_outs:
                nc.sync.dma_start(dbg["d_a0"].ap(), apad[0][:])
                nc.sync.dma_start(dbg["d_ag0"].ap(), ag[0][:])

            # ---- fc2 + final residuals, pipelined per M-tile ----
            # psum = fc2(ag) + x_attn (identity matmul); u = g2*z2 + bfin on
            # ScalarE; y = u + psum; per-mt output DMA overlaps mt=1 matmuls.
            fc2_ps = [psS.tile([128, N], F32, tag="psS", name=f"fc2_{mt}")
                      for mt in range(2)]
            for kt in range(2):
                npart = nparts[kt]
                for mt in range(2):
                    for c in range(2):
                        sl = slice(c * 512, (c + 1) * 512)
                        nc.tensor.matmul(
                            fc2_ps[mt][:, sl],
                            consts["wfc2T"][0:npart, kt, mt * 128:(mt + 1) * 128],
                            ag[kt][0:npart, sl], start=(kt == 0), stop=(kt == 1))
            for mt in range(2):
                ps = fc2_ps[mt]
                for c in range(2):
                    sl = slice(c * 512, (c + 1) * 512)
                    nc.tensor.matmul(ps[:, sl], consts["id128"][:], x_attn[mt][:, sl],
                                     start=False, stop=True, skip_group_check=True)
                ut = work.tile([128, N], BF16, tag=f"u{mt}", name=f"u{mt}")
                nc.scalar.activation(ut[:], z2[mt][:], AF.Identity,
                                     bias=col(f"bfin{mt}"), scale=col(f"g2c{mt}"))
                yt = work.tile([128, N], F32, tag=f"y{mt}", name=f"y{mt}")
                nc.vector.tensor_tensor(yt[:], ut[:], ps[:], OP.add)
                nc.sync.dma_start(y_d.ap()[mt * 128:(mt + 1) * 128, :], yt[:])
                if mt == 0:
                    sdummy(ut[:, 0:128], ut[:, 0:512])

    nc.compile()
    return nc


_NC = None


def kernel(**inputs):
    global _NC
    consts = fold_consts(inputs)
    if _NC is None:
        _NC = build()
    x = np.asarray(inputs["x"], np.float32)
    B = x.shape[0]
    in_maps = []
    for b in range(B):
        m = dict(consts)
        m["x"] = pack_x(x[b].reshape(C, N))
        in_maps.append(m)
    res = run_bass_kernel_spmd(_NC, in_maps, core_ids=list(range(B)))
    out = np.stack([res.results[b]["y"].reshape(C, HH, WW) for b in range(B)])
    return out
